# revision 2
# baseline (speedup 1.0000x reference)
# Trainium2 Bass kernel v2 for relative-position causal attention.
#
# Sharding: core = (z, head-quad): z = core//4, heads [4*(core%4), 4*(core%4)+4).
# Each core computes its z's x-projections for its 4 heads, attention, and a
# partial output projection [D, N] fp16; host sums 4 partials per z.
#
# Precision: every fp32 matmul is replaced by fp16 hi/lo splits.
#  - projections (contraction 128/chunk): 3 fp16 matmuls  Wh*xh + Wh*xl + Wl*xh
#  - logits (contraction 64/head): 2 fp16 matmuls
#        qh*kh          (64-deep)
#      + [qh;ql]*[kl;kh] (128-deep packed cross terms)
#    accumulated in fp32 PSUM. Dropped ll terms are ~2^-24 relative.
# The position table T[delta,hd] = fpe @ sincos is input-independent and is
# precomputed on host (fp64), shipped as packed hi/lo fp16.
#
# Position logits are computed in diagonal layout (a plain matmul against the
# table) then sheared to row layout via a DRAM strip with per-row pitch W+128;
# columns [W, W+128) of the strip hold -1e30 so the out-of-causal reads that
# the shear produces come back as -inf-like logits: the causal mask is free.
# Content psum + sheared position are fused with one DVE tensor_tensor_reduce
# per chunk that also produces the row max for the softmax.

from contextlib import ExitStack

import numpy as np

N = 2048
M = 2048
B = 2
D = 1024
H = 16
DQK = 64
DV = 64
NCORES = 8
HQ = 4  # heads per core
KT = 8  # contraction chunks over D
TW = 2048  # table diagonals
TPAD = 128  # strip pad width (-1e30)

_cache = {}


def _build(loop=1):
    import concourse.bacc as bacc
    import concourse.mybir as mybir
    import concourse.tile as tile
    from concourse.bass import AP
    from concourse.masks import make_identity
    from concourse.tile_rust import add_dep_helper

    f32 = mybir.dt.float32
    f16 = mybir.dt.float16
    AX = mybir.AxisListType.X
    ADD = mybir.AluOpType.add
    MAX = mybir.AluOpType.max
    EXP = mybir.ActivationFunctionType.Exp

    nc = bacc.Bacc("TRN2", target_bir_lowering=False, debug=False, num_devices=NCORES)

    xqh_d = nc.dram_tensor("xqh", [D, N], f16, kind="ExternalInput")
    xql_d = nc.dram_tensor("xql", [D, N], f16, kind="ExternalInput")
    xkvh_d = nc.dram_tensor("xkvh", [D, N], f16, kind="ExternalInput")
    xkvl_d = nc.dram_tensor("xkvl", [D, N], f16, kind="ExternalInput")
    tab_d = nc.dram_tensor("tabpk", [128, HQ * TW], f16, kind="ExternalInput")
    wqh_d = nc.dram_tensor("wqh", [D, 256], f16, kind="ExternalInput")
    wql_d = nc.dram_tensor("wql", [D, 256], f16, kind="ExternalInput")
    wkh_d = nc.dram_tensor("wkh", [D, 256], f16, kind="ExternalInput")
    wkl_d = nc.dram_tensor("wkl", [D, 256], f16, kind="ExternalInput")
    wv_d = nc.dram_tensor("wv", [D, 256], f16, kind="ExternalInput")
    wo_d = nc.dram_tensor("wo", [256, D], f16, kind="ExternalInput")
    outT = nc.dram_tensor("outT", [8, 128, N], f16, kind="ExternalOutput")

    # shear strip buffers: uniform pitch 2176, data right-aligned at col 2048,
    # pad [-1e30] at cols [2048, 2176) written once per buffer
    NSCR = 16
    SPITCH = 2048 + TPAD
    scrb = [
        nc.dram_tensor(f"scr{j}", [128 * SPITCH], f32, kind="Internal")
        for j in range(NSCR)
    ]
    # batched strips for small blocks: one tensor per i<8 with 4 regions
    scrs = {
        i: nc.dram_tensor(
            f"scs{i}", [4 * 128 * (128 * (i + 1) + TPAD)], f32, kind="Internal"
        )
        for i in range(8)
    }

    with tile.TileContext(nc) as tc:
        with ExitStack() as ctx:
            wpool = ctx.enter_context(tc.tile_pool(name="wpool", bufs=1))
            big = ctx.enter_context(tc.tile_pool(name="big", bufs=1))
            work = ctx.enter_context(tc.tile_pool(name="work", bufs=4))

            ident16 = wpool.tile([128, 128], f16)
            make_identity(nc, ident16[:])
            wqh_sb = wpool.tile([128, KT, 256], f16)
            wql_sb = wpool.tile([128, KT, 256], f16)
            wkh_sb = wpool.tile([128, KT, 256], f16)
            wkl_sb = wpool.tile([128, KT, 256], f16)
            wv_sb = wpool.tile([128, KT, 256], f16)
            wo_sb = wpool.tile([128, 2, D], f16)
            tab_sb = wpool.tile([128, HQ, TW], f16)
            nc.sync.dma_start(
                wqh_sb[:], wqh_d.ap().rearrange("(t p) m -> p t m", p=128)
            )

            padc = wpool.tile([128, 4 * TPAD], f32)
            nc.vector.memset(padc[:], -1.0e30)
            pad_insts = []
            last_read = [None] * NSCR
            next_scr = [0]
            pad_s_insts = {}
            last_read_s = {}

            # persistent activations
            qpk = big.tile([128, HQ, N], f16)  # per head: rows [qh; ql]
            qdup = big.tile([128, HQ, N], f16)  # rows [unused; qh] (base-64 qh)
            kpk = big.tile([128, HQ, M], f16)  # per head: rows [kl; kh]
            V_sb = big.tile([128, 16, 256], f16)  # [m-part, m-block, hv]
            attn16 = big.tile([128, 2, N], f16)  # [hv-group rows, g, n]

            def proj_split_repack(xp, xs_pool, x_hi_tiles, x_lo_loader, wh, wl, pk, hi_top):
                # 3-matmul hi/lo projection of [256, N]; split into fp16 hi/lo
                # and repack per-head into pk ([hi;lo] or [lo;hi] stacking).
                with tc.tile_pool(name="pp", bufs=1, space="PSUM") as pp:
                    ps = [
                        pp.tile([128, 512], f32, tag=f"p{j}", name=f"p{j}")
                        for j in range(8)
                    ]
                    for t in range(KT):
                        xh_t = x_hi_tiles(t)
                        xl_t = x_lo_loader(t)
                        for g in range(2):
                            for cc in range(4):
                                p = ps[g * 4 + cc]
                                wslc = slice(128 * g, 128 * (g + 1))
                                xs = slice(512 * cc, 512 * (cc + 1))
                                nc.tensor.matmul(
                                    p[:], wh[:, t, wslc], xh_t[:, xs],
                                    start=(t == 0), stop=False,
                                )
                                nc.tensor.matmul(
                                    p[:], wh[:, t, wslc], xl_t[:, xs],
                                    start=False, stop=False,
                                )
                                nc.tensor.matmul(
                                    p[:], wl[:, t, wslc], xh_t[:, xs],
                                    start=False, stop=(t == KT - 1),
                                )
                    sp_h = xs_pool.tile([128, 2, N], f16, tag="sph")
                    sp_l = xs_pool.tile([128, 2, N], f16, tag="spl")
                    for g in range(2):
                        for cc in range(4):
                            p = ps[g * 4 + cc]
                            xs = slice(512 * cc, 512 * (cc + 1))
                            nc.scalar.copy(sp_h[:, g, xs], p[:])
                            nc.vector.tensor_sub(sp_l[:, g, xs], p[:], sp_h[:, g, xs])
                    top, bot = (sp_h, sp_l) if hi_top else (sp_l, sp_h)
                    for h in range(HQ):
                        rs = slice(64 * (h % 2), 64 * (h % 2) + 64)
                        nc.sync.dma_start(pk[0:64, h, :], top[rs, h // 2, :])
                        nc.sync.dma_start(pk[64:128, h, :], bot[rs, h // 2, :])
                        if hi_top:  # q: also duplicate qh at base partition 64
                            nc.sync.dma_start(qdup[64:128, h, :], sp_h[rs, h // 2, :])

            def phase_kqv(xp, xs_pool, xkv_pool):
                # prefetch t=0 x tiles before the wql load so the first
                # matmuls of the q projection start as early as possible
                xh0 = xp.tile([128, N], f16, tag="xhi")
                nc.sync.dma_start(
                    xh0[:], xqh_d.ap().rearrange("(t p) n -> p t n", p=128)[:, 0, :]
                )
                xl0 = xp.tile([128, N], f16, tag="xlo")
                nc.sync.dma_start(
                    xl0[:], xql_d.ap().rearrange("(t p) n -> p t n", p=128)[:, 0, :]
                )
                nc.sync.dma_start(
                    wql_sb[:], wql_d.ap().rearrange("(t p) m -> p t m", p=128)
                )

                def q_hi(t):
                    if t == 0:
                        return xh0
                    xh = xp.tile([128, N], f16, tag="xhi")
                    nc.sync.dma_start(
                        xh[:], xqh_d.ap().rearrange("(t p) n -> p t n", p=128)[:, t, :]
                    )
                    return xh

                def q_lo(t):
                    if t == 0:
                        return xl0
                    xl = xp.tile([128, N], f16, tag="xlo")
                    nc.sync.dma_start(
                        xl[:], xql_d.ap().rearrange("(t p) n -> p t n", p=128)[:, t, :]
                    )
                    return xl

                nc.sync.dma_start(
                    tab_sb[:], tab_d.ap().rearrange("p (h t) -> p h t", h=HQ)
                )
                proj_split_repack(
                    xp, xs_pool, q_hi, q_lo, wqh_sb, wql_sb, qpk, hi_top=True
                )

                for dst, srct in [(wkh_sb, wkh_d), (wkl_sb, wkl_d)]:
                    nc.sync.dma_start(
                        dst[:], srct.ap().rearrange("(t p) m -> p t m", p=128)
                    )
                xkvh_sb = xkv_pool.tile([128, KT, N], f16)
                kv_loaded = set()

                def kv_hi(t):
                    if t not in kv_loaded:
                        kv_loaded.add(t)
                        nc.sync.dma_start(
                            xkvh_sb[:, t, :],
                            xkvh_d.ap().rearrange("(t p) n -> p t n", p=128)[
                                :, t, :
                            ],
                        )
                    return xkvh_sb[:, t, :]

                def kv_lo(t):
                    xl = xp.tile([128, N], f16, tag="xlo")
                    nc.sync.dma_start(
                        xl[:], xkvl_d.ap().rearrange("(t p) n -> p t n", p=128)[:, t, :]
                    )
                    return xl

                proj_split_repack(
                    xp, xs_pool, kv_hi, kv_lo, wkh_sb, wkl_sb, kpk, hi_top=False
                )

                # V directly in [m, hv] layout: V = x_t.T @ wv (fp16)
                nc.sync.dma_start(
                    wv_sb[:], wv_d.ap().rearrange("(t p) m -> p t m", p=128)
                )
                with tc.tile_pool(name="pv", bufs=1, space="PSUM") as pv:
                    for half in range(2):
                        vps = [
                            pv.tile([128, 256], f32, tag=f"v{j}", name=f"v{j}")
                            for j in range(8)
                        ]
                        for t in range(KT):
                            for j in range(8):
                                mb = 8 * half + j
                                nc.tensor.matmul(
                                    vps[j][:],
                                    xkvh_sb[:, t, 128 * mb : 128 * (mb + 1)],
                                    wv_sb[:, t, :],
                                    start=(t == 0), stop=(t == KT - 1),
                                )
                        for j in range(8):
                            nc.vector.tensor_copy(V_sb[:, 8 * half + j, :], vps[j][:])
                nc.sync.dma_start(
                    wo_sb[:], wo_d.ap().rearrange("(g p) d -> p g d", p=128)
                )
                for j in range(NSCR):
                    pad_insts.append(
                        nc.gpsimd.dma_start(
                            AP(scrb[j], 2048, [[SPITCH, 128], [1, TPAD]]),
                            padc[:, :TPAD],
                        )
                    )
                for i in range(8):
                    W = 128 * (i + 1)
                    P = W + TPAD
                    pad_s_insts[i] = nc.gpsimd.dma_start(
                        AP(scrs[i], W, [[P, 128], [128 * P, 4], [1, TPAD]]),
                        padc[:],
                    )

            def strip_mms(i, h, tT_ap, col0, tps_pool):
                # diagonal-layout position strip matmuls for (h, block i) into
                # tT_ap columns [col0, col0+W)
                q_hh = qdup[64:128, h, :]
                q_pk = qpk[:, h, :]
                t_hh = tab_sb[64:128, h, :]
                t_pk = tab_sb[:, h, :]
                nb = 128 * i
                W = nb + 128
                nslc = slice(nb, nb + 128)
                for sc in range((W + 511) // 512):
                    wdt = min(512, W - 512 * sc)
                    tps = tps_pool.tile([128, 512], f32, tag="tps")
                    ts = slice(TW - W + 512 * sc, TW - W + 512 * sc + wdt)
                    nc.tensor.matmul(
                        tps[:, :wdt], q_hh[:, nslc], t_hh[:, ts],
                        start=True, stop=False,
                    )
                    nc.tensor.matmul(
                        tps[:, :wdt], q_pk[:, nslc], t_pk[:, ts],
                        start=False, stop=True,
                    )
                    cs = col0 + 512 * sc
                    nc.scalar.copy(tT_ap[:, cs : cs + wdt], tps[:, :wdt])

            def emit_strips(i, tps_pool, lpool, tpool):
                # returns {h: (Lp_tile, col0)}; batches 4/2/1 heads per DRAM
                # round trip depending on the strip width
                W = 128 * (i + 1)
                P = W + TPAD
                nbh = 4 if i < 4 else (2 if i < 8 else 1)
                out = {}
                for tb_ in range(HQ // nbh):
                    heads = [tb_ * nbh + j for j in range(nbh)]
                    tT = tpool.tile([128, 2048], f32, tag="tT")
                    for idx, h in enumerate(heads):
                        strip_mms(i, h, tT, idx * W, tps_pool)
                    if nbh == 1:
                        h = heads[0]
                        bj = next_scr[0] % NSCR
                        next_scr[0] += 1
                        s = scrb[bj]
                        w_inst = nc.sync.dma_start(
                            AP(s, 2048 - W, [[SPITCH, 128], [1, W]]), tT[:, :W]
                        )
                        if last_read[bj] is not None:
                            add_dep_helper(
                                w_inst.ins, last_read[bj].ins, reason="WAR"
                            )
                        Lp = lpool.tile([128, 2048], f32, tag="Lp")
                        r_inst = nc.sync.dma_start(
                            Lp[:, :W],
                            AP(s, 2048 - W + 127, [[SPITCH - 1, 128], [1, W]]),
                        )
                        add_dep_helper(r_inst.ins, w_inst.ins, reason="RAW")
                        add_dep_helper(r_inst.ins, pad_insts[bj].ins, reason="pad")
                        last_read[bj] = r_inst
                        out[h] = (Lp, 0)
                    else:
                        s = scrs[i]
                        base = heads[0] * 128 * P
                        w_inst = nc.sync.dma_start(
                            AP(s, base, [[P, 128], [128 * P, nbh], [1, W]]),
                            tT[:, : nbh * W],
                        )
                        key = (i, tb_)
                        if last_read_s.get(key) is not None:
                            add_dep_helper(
                                w_inst.ins, last_read_s[key].ins, reason="WAR"
                            )
                        Lp = lpool.tile([128, 2048], f32, tag="Lp")
                        r_inst = nc.sync.dma_start(
                            Lp[:, : nbh * W],
                            AP(
                                s, base + 127,
                                [[P - 1, 128], [128 * P, nbh], [1, W]],
                            ),
                        )
                        add_dep_helper(r_inst.ins, w_inst.ins, reason="RAW")
                        add_dep_helper(
                            r_inst.ins, pad_s_insts[i].ins, reason="pad"
                        )
                        last_read_s[key] = r_inst
                        for idx, h in enumerate(heads):
                            out[h] = (Lp, idx * W)
                return out

            def phase_attn(pa, pb, pc, lpool, lpool3, tpool, spool):
                done = set()
                for i in range(16):
                    nb = 128 * i
                    W = nb + 128
                    nch = (W + 511) // 512
                    nslc = slice(nb, nb + 128)
                    lps = emit_strips(i, pa, lpool3, tpool)
                    for h in range(HQ):
                        q_hh = qdup[64:128, h, :]
                        q_pk = qpk[:, h, :]
                        k_hh = kpk[64:128, h, :]
                        k_pk = kpk[:, h, :]
                        Lp, col0 = lps[h]

                        # --- content logits + fused add/max ---
                        if i < 4:
                            L_sb = spool.tile([128, 512], f32, tag="Ls")
                        else:
                            L_sb = lpool.tile([128, 2048], f32, tag="L")
                        cmax = work.tile([128, 4], f32, tag="cmax")
                        for c in range(nch):
                            rw = min(512, W - 512 * c)
                            ms = slice(512 * c, 512 * c + rw)
                            cps = pb.tile([128, 512], f32, tag="cps")
                            nc.tensor.matmul(
                                cps[:, :rw], q_hh[:, nslc], k_hh[:, ms],
                                start=True, stop=False,
                            )
                            nc.tensor.matmul(
                                cps[:, :rw], q_pk[:, nslc], k_pk[:, ms],
                                start=False, stop=True,
                            )
                            nc.vector.tensor_add(
                                L_sb[:, ms],
                                cps[:, :rw],
                                Lp[:, col0 + 512 * c : col0 + 512 * c + rw],
                            )
                            if nch > 1:
                                nc.vector.reduce_max(
                                    out=cmax[:, c : c + 1], in_=L_sb[:, ms], axis=AX
                                )
                        negmax = work.tile([128, 1], f32, tag="negmax")
                        if nch == 1:
                            nc.vector.tensor_reduce(
                                out=negmax[:], in_=L_sb[:, :W], axis=AX, op=MAX,
                                negate=True,
                            )
                        else:
                            nc.vector.tensor_reduce(
                                out=negmax[:], in_=cmax[:, :nch], axis=AX, op=MAX,
                                negate=True,
                            )

                        # --- softmax ---
                        if i < 4:
                            P_all = spool.tile([128, 512], f16, tag="Ps")
                        else:
                            P_all = tpool.tile([128, 2048], f16, tag="Pall")
                        ssum = work.tile([128, 1], f32, tag="ssum")
                        nc.scalar.activation(
                            P_all[:, :W], L_sb[:, :W], EXP,
                            bias=negmax[:], scale=1.0, accum_out=ssum[:],
                        )
                        rsum = work.tile([128, 1], f32, tag="rsum")
                        nc.vector.reciprocal(rsum[:], ssum[:])
                        nc.vector.tensor_scalar_mul(
                            P_all[:, :W], P_all[:, :W], rsum[:]
                        )

                        # --- transpose P + AV ---
                        atps = pc.tile([64, 128], f32, tag="avp")
                        for tb in range((i + 8) // 8):
                            nsub = min(8, i + 1 - 8 * tb)
                            ptps = pa.tile([128, 1024], f16, tag="ptp")
                            for sb_ in range(nsub):
                                mt = 8 * tb + sb_
                                nc.tensor.transpose(
                                    ptps[:, 128 * sb_ : 128 * (sb_ + 1)],
                                    P_all[:, 128 * mt : 128 * (mt + 1)],
                                    ident16[:],
                                )
                            pt_sb = work.tile([128, 1024], f16, tag="pts")
                            if i >= 10 or (i + tb) % 2 == 0:
                                nc.scalar.copy(
                                    pt_sb[:, : 128 * nsub], ptps[:, : 128 * nsub]
                                )
                            else:
                                nc.vector.tensor_copy(
                                    pt_sb[:, : 128 * nsub], ptps[:, : 128 * nsub]
                                )
                            for sb_ in range(nsub):
                                mt = 8 * tb + sb_
                                nc.tensor.matmul(
                                    atps[:],
                                    V_sb[:, mt, 64 * h : 64 * (h + 1)],
                                    pt_sb[:, 128 * sb_ : 128 * (sb_ + 1)],
                                    start=(mt == 0), stop=(mt == i),
                                )
                        ro = 64 * (h % 2)
                        if i >= 10:
                            nc.scalar.copy(
                                attn16[ro : ro + 64, h // 2, nslc], atps[:]
                            )
                        else:
                            nc.vector.tensor_copy(
                                attn16[ro : ro + 64, h // 2, nslc], atps[:]
                            )
                    # incremental out projection per 512-column chunk
                    done.add(i)
                    nn = i // 4
                    if all(4 * nn + j in done for j in range(4)):
                        phase_out_chunk(pb, nn)

            def phase_out_chunk(pa, nn):
                ns = slice(512 * nn, 512 * (nn + 1))
                for dc in range(8):
                    ops = pa.tile([128, 512], f32, tag="cps")
                    dslc = slice(128 * dc, 128 * (dc + 1))
                    nc.tensor.matmul(
                        ops[:], wo_sb[:, 0, dslc], attn16[:, 0, ns],
                        start=True, stop=False,
                    )
                    nc.tensor.matmul(
                        ops[:], wo_sb[:, 1, dslc], attn16[:, 1, ns],
                        start=False, stop=True,
                    )
                    o_sb = work.tile([128, 512], f16, tag="osb")
                    nc.scalar.copy(o_sb[:], ops[:])
                    nc.sync.dma_start(outT.ap()[dc, :, ns], o_sb[:])

            def loop_body():
                with ExitStack() as phx:
                    xp = phx.enter_context(tc.tile_pool(name="xp", bufs=3))
                    xs_pool = phx.enter_context(tc.tile_pool(name="xs", bufs=1))
                    xkv_pool = phx.enter_context(tc.tile_pool(name="xkv", bufs=1))
                    phase_kqv(xp, xs_pool, xkv_pool)
                with ExitStack() as phx:
                    lpool = phx.enter_context(tc.tile_pool(name="lpool", bufs=2))
                    lpool3 = phx.enter_context(tc.tile_pool(name="lpool3", bufs=2))
                    tpool = phx.enter_context(tc.tile_pool(name="tpool", bufs=3))
                    pa = phx.enter_context(tc.tile_pool(name="pa", bufs=2, space="PSUM"))
                    pb = phx.enter_context(tc.tile_pool(name="pb", bufs=3, space="PSUM"))
                    pc = phx.enter_context(tc.tile_pool(name="pc", bufs=1, space="PSUM"))
                    spool = phx.enter_context(tc.tile_pool(name="spool", bufs=4))
                    phase_attn(pa, pb, pc, lpool, lpool3, tpool, spool)

            if loop == 1:
                loop_body()
            else:
                with tc.For_i(0, loop, 1):
                    loop_body()

    nc.compile()
    return nc


def _host_table():
    """Position table T[delta, h, dqk] for delta in [0, 2048), fp32 sincos
    pipeline (matching the reference) with an fp64 contraction."""
    r = np.arange(0.0, float(TW), dtype=np.float32)
    inv_freq = (
        1.0 / (10000.0 ** (np.arange(0.0, D, 2.0, dtype=np.float32) / np.float32(D)))
    ).astype(np.float32)
    phases = (r[:, None] * inv_freq[None, :]).astype(np.float32)
    sc = np.concatenate(
        [np.sin(phases, dtype=np.float32), np.cos(phases, dtype=np.float32)], axis=-1
    )
    return sc  # [TW, D]


def _split16(a):
    h = a.astype(np.float16)
    l = (a - h.astype(np.float32)).astype(np.float16)
    return h, l


def _prep_inputs(x_q, x_kv, to_q, to_kv, for_pos_enc, to_o):
    x_q = np.asarray(x_q, dtype=np.float32)
    x_kv = np.asarray(x_kv, dtype=np.float32)
    to_q = np.asarray(to_q, dtype=np.float32)
    to_kv = np.asarray(to_kv, dtype=np.float32)
    fpe = np.asarray(for_pos_enc, dtype=np.float32)
    to_o = np.asarray(to_o, dtype=np.float32)

    if "sc" not in _cache:
        _cache["sc"] = _host_table()
    sc = _cache["sc"]
    # T[delta, h, dqk] in fp64 then reversed over delta
    T = np.einsum("rb,hdb->rhd", sc.astype(np.float64), fpe.astype(np.float64))
    T = np.ascontiguousarray(T[::-1]).astype(np.float32)  # col jr <-> delta 2047-jr

    xs = {}
    for z in range(B):
        xs[("q", z)] = _split16(np.ascontiguousarray(x_q[:, z, :].T))
        xs[("kv", z)] = _split16(np.ascontiguousarray(x_kv[:, z, :].T))

    in_maps = []
    for c in range(NCORES):
        z = c // 4
        quad = c % 4
        hs = slice(HQ * quad, HQ * (quad + 1))
        wq = np.ascontiguousarray(to_q[hs].reshape(HQ * DQK, D).T)
        wk = np.ascontiguousarray(to_kv[hs, :DQK].reshape(HQ * DQK, D).T)
        wqh, wql = _split16(wq)
        wkh, wkl = _split16(wk)
        wv = np.ascontiguousarray(to_kv[hs, DQK:].reshape(HQ * DV, D).T).astype(
            np.float16
        )
        wo = np.ascontiguousarray(
            (to_o[:, hs, :] / 16.0).reshape(D, HQ * DV).T
        ).astype(np.float16)
        # packed table: per head rows [tabl(64); tabh(64)], cols reversed delta
        tab = np.empty((128, HQ * TW), dtype=np.float16)
        for h in range(HQ):
            Th, Tl = _split16(np.ascontiguousarray(T[:, HQ * quad + h, :].T))  # [64,TW]
            tab[0:64, TW * h : TW * (h + 1)] = Tl
            tab[64:128, TW * h : TW * (h + 1)] = Th
        xqh, xql = xs[("q", z)]
        xkvh, xkvl = xs[("kv", z)]
        in_maps.append(
            {
                "xqh": xqh, "xql": xql, "xkvh": xkvh, "xkvl": xkvl,
                "tabpk": tab,
                "wqh": wqh, "wql": wql, "wkh": wkh, "wkl": wkl,
                "wv": wv, "wo": wo,
            }
        )
    return in_maps


def kernel(x_q, x_kv, to_q, to_kv, for_pos_enc, to_o):
    from concourse.bass_utils import run_bass_kernel_spmd

    if "nc" not in _cache:
        _cache["nc"] = _build()
    nc = _cache["nc"]
    in_maps = _prep_inputs(x_q, x_kv, to_q, to_kv, for_pos_enc, to_o)
    res = run_bass_kernel_spmd(nc, in_maps, core_ids=list(range(NCORES)))
    out = np.zeros((B, D, N), dtype=np.float64)
    for c in range(NCORES):
        z = c // 4
        out[z] += res.results[c]["outT"].reshape(D, N).astype(np.float64)
    out *= 16.0
    return np.ascontiguousarray(out.transpose(2, 0, 1)).astype(np.float32)


# revision 17
# speedup vs baseline: 1.0334x; 1.0334x over previous
# Trainium2 Bass kernel v2 for relative-position causal attention.
#
# Sharding: core = (z, head-quad): z = core//4, heads [4*(core%4), 4*(core%4)+4).
# Each core computes its z's x-projections for its 4 heads, attention, and a
# partial output projection [D, N] fp16; host sums 4 partials per z.
#
# Precision: every fp32 matmul is replaced by fp16 hi/lo splits.
#  - projections (contraction 128/chunk): 3 fp16 matmuls  Wh*xh + Wh*xl + Wl*xh
#  - logits (contraction 64/head): 2 fp16 matmuls
#        qh*kh          (64-deep)
#      + [qh;ql]*[kl;kh] (128-deep packed cross terms)
#    accumulated in fp32 PSUM. Dropped ll terms are ~2^-24 relative.
# The position table T[delta,hd] = fpe @ sincos is input-independent and is
# precomputed on host (fp64), shipped as packed hi/lo fp16.
#
# Position logits are computed in diagonal layout (a plain matmul against the
# table) then sheared to row layout via a DRAM strip with per-row pitch W+128;
# columns [W, W+128) of the strip hold -1e30 so the out-of-causal reads that
# the shear produces come back as -inf-like logits: the causal mask is free.
# Content psum + sheared position are fused with one DVE tensor_tensor_reduce
# per chunk that also produces the row max for the softmax.

from contextlib import ExitStack

import numpy as np

N = 2048
M = 2048
B = 2
D = 1024
H = 16
DQK = 64
DV = 64
NCORES = 8
HQ = 4  # heads per core
KT = 8  # contraction chunks over D
TW = 2048  # table diagonals
TPAD = 128  # strip pad width (-1e30)

_cache = {}


def _build(loop=1):
    import concourse.bacc as bacc
    import concourse.mybir as mybir
    import concourse.tile as tile
    from concourse.bass import AP
    from concourse.masks import make_identity
    from concourse.tile_rust import add_dep_helper

    f32 = mybir.dt.float32
    f16 = mybir.dt.float16
    AX = mybir.AxisListType.X
    ADD = mybir.AluOpType.add
    MAX = mybir.AluOpType.max
    EXP = mybir.ActivationFunctionType.Exp

    nc = bacc.Bacc("TRN2", target_bir_lowering=False, debug=False, num_devices=NCORES)

    xqh_d = nc.dram_tensor("xqh", [D, N], f16, kind="ExternalInput")
    xql_d = nc.dram_tensor("xql", [D, N], f16, kind="ExternalInput")
    xkvh_d = nc.dram_tensor("xkvh", [D, N], f16, kind="ExternalInput")
    xkvl_d = nc.dram_tensor("xkvl", [D, N], f16, kind="ExternalInput")
    tab_d = nc.dram_tensor("tabpk", [128, HQ * TW], f16, kind="ExternalInput")
    wqh_d = nc.dram_tensor("wqh", [D, 256], f16, kind="ExternalInput")
    wql_d = nc.dram_tensor("wql", [D, 256], f16, kind="ExternalInput")
    wkh_d = nc.dram_tensor("wkh", [D, 256], f16, kind="ExternalInput")
    wkl_d = nc.dram_tensor("wkl", [D, 256], f16, kind="ExternalInput")
    wv_d = nc.dram_tensor("wv", [D, 256], f16, kind="ExternalInput")
    wo_d = nc.dram_tensor("wo", [256, D], f16, kind="ExternalInput")
    outT = nc.dram_tensor("outT", [8, 128, N], f16, kind="ExternalOutput")

    # shear strip buffers: uniform pitch 2176, data right-aligned at col 2048,
    # pad [-1e30] at cols [2048, 2176) written once per buffer
    NSCR = 16
    SPITCH = 2048 + TPAD
    scrb = [
        nc.dram_tensor(f"scr{j}", [128 * SPITCH], f32, kind="Internal")
        for j in range(NSCR)
    ]
    # batched strips for small blocks: one tensor per i<8 with 4 regions
    scrs = {
        i: nc.dram_tensor(
            f"scs{i}", [4 * 128 * (128 * (i + 1) + TPAD)], f32, kind="Internal"
        )
        for i in range(8)
    }

    with tile.TileContext(nc) as tc:
        with ExitStack() as ctx:
            wpool = ctx.enter_context(tc.tile_pool(name="wpool", bufs=1))
            big = ctx.enter_context(tc.tile_pool(name="big", bufs=1))
            work = ctx.enter_context(tc.tile_pool(name="work", bufs=4))

            ident16 = wpool.tile([128, 128], f16)
            make_identity(nc, ident16[:])
            wqh_sb = wpool.tile([128, KT, 256], f16)
            wql_sb = wpool.tile([128, KT, 256], f16)
            wkh_sb = wpool.tile([128, KT, 256], f16)
            wkl_sb = wpool.tile([128, KT, 256], f16)
            wv_sb = wpool.tile([128, KT, 256], f16)
            wo_sb = wpool.tile([128, 2, D], f16)
            tab_sb = wpool.tile([128, HQ, TW], f16)
            nc.sync.dma_start(
                wqh_sb[:, 0, :],
                wqh_d.ap().rearrange("(t p) m -> p t m", p=128)[:, 0, :],
            )

            padc = wpool.tile([128, 4 * TPAD], f32)
            nc.vector.memset(padc[:], -1.0e30)
            pad_insts = []
            last_read = [None] * NSCR
            next_scr = [0]
            pad_s_insts = {}
            last_read_s = {}

            # persistent activations
            qpk = big.tile([128, HQ, N], f16)  # per head: rows [qh; ql]
            qdup = big.tile([128, HQ, N], f16)  # rows [unused; qh] (base-64 qh)
            kpk = big.tile([128, HQ, M], f16)  # per head: rows [kl; kh]
            V_sb = big.tile([128, 16, 256], f16)  # [m-part, m-block, hv]
            attn16 = big.tile([128, 2, N], f16)  # [hv-group rows, g, n]

            def proj_split_repack(xp, xs_pool, x_hi_tiles, x_lo_loader, wh, wl, pk, hi_top):
                # 3-matmul hi/lo projection of [256, N]; split into fp16 hi/lo
                # and repack per-head into pk ([hi;lo] or [lo;hi] stacking).
                with tc.tile_pool(name="pp", bufs=1, space="PSUM") as pp:
                    ps = [
                        pp.tile([128, 512], f32, tag=f"p{j}", name=f"p{j}")
                        for j in range(8)
                    ]
                    for t in range(KT):
                        xh_t = x_hi_tiles(t)
                        xl_t = x_lo_loader(t)
                        for g in range(2):
                            for cc in range(4):
                                p = ps[g * 4 + cc]
                                wslc = slice(128 * g, 128 * (g + 1))
                                xs = slice(512 * cc, 512 * (cc + 1))
                                nc.tensor.matmul(
                                    p[:], wh[:, t, wslc], xh_t[:, xs],
                                    start=(t == 0), stop=False,
                                )
                                nc.tensor.matmul(
                                    p[:], wh[:, t, wslc], xl_t[:, xs],
                                    start=False, stop=False,
                                )
                                nc.tensor.matmul(
                                    p[:], wl[:, t, wslc], xh_t[:, xs],
                                    start=False, stop=(t == KT - 1),
                                )
                    sp_h = xs_pool.tile([128, 2, N], f16, tag="sph")
                    sp_l = xs_pool.tile([128, 2, N], f16, tag="spl")
                    for g in range(2):
                        for cc in range(4):
                            p = ps[g * 4 + cc]
                            xs = slice(512 * cc, 512 * (cc + 1))
                            nc.scalar.copy(sp_h[:, g, xs], p[:])
                            nc.vector.tensor_sub(sp_l[:, g, xs], p[:], sp_h[:, g, xs])
                    top, bot = (sp_h, sp_l) if hi_top else (sp_l, sp_h)
                    for h in range(HQ):
                        rs = slice(64 * (h % 2), 64 * (h % 2) + 64)
                        nc.sync.dma_start(pk[0:64, h, :], top[rs, h // 2, :])
                        nc.sync.dma_start(pk[64:128, h, :], bot[rs, h // 2, :])
                        if hi_top:  # q: also duplicate qh at base partition 64
                            nc.sync.dma_start(qdup[64:128, h, :], sp_h[rs, h // 2, :])

            def phase_kqv(xp, xs_pool, xkv_pool):
                # prefetch t=0 x tiles before the wql load so the first
                # matmuls of the q projection start as early as possible
                xh0 = xp.tile([128, N], f16, tag="xhi")
                nc.sync.dma_start(
                    xh0[:], xqh_d.ap().rearrange("(t p) n -> p t n", p=128)[:, 0, :]
                )
                xl0 = xp.tile([128, N], f16, tag="xlo")
                nc.sync.dma_start(
                    xl0[:], xql_d.ap().rearrange("(t p) n -> p t n", p=128)[:, 0, :]
                )
                nc.sync.dma_start(
                    wql_sb[:, 0, :],
                    wql_d.ap().rearrange("(t p) m -> p t m", p=128)[:, 0, :],
                )
                for t in range(1, KT):
                    for dst, srct in [(wqh_sb, wqh_d), (wql_sb, wql_d)]:
                        nc.sync.dma_start(
                            dst[:, t, :],
                            srct.ap().rearrange("(t p) m -> p t m", p=128)[:, t, :],
                        )

                def q_hi(t):
                    if t == 0:
                        return xh0
                    xh = xp.tile([128, N], f16, tag="xhi")
                    nc.sync.dma_start(
                        xh[:], xqh_d.ap().rearrange("(t p) n -> p t n", p=128)[:, t, :]
                    )
                    return xh

                def q_lo(t):
                    if t == 0:
                        return xl0
                    xl = xp.tile([128, N], f16, tag="xlo")
                    nc.sync.dma_start(
                        xl[:], xql_d.ap().rearrange("(t p) n -> p t n", p=128)[:, t, :]
                    )
                    return xl

                proj_split_repack(
                    xp, xs_pool, q_hi, q_lo, wqh_sb, wql_sb, qpk, hi_top=True
                )
                nc.sync.dma_start(
                    tab_sb[:], tab_d.ap().rearrange("p (h t) -> p h t", h=HQ)
                )

                for dst, srct in [(wkh_sb, wkh_d), (wkl_sb, wkl_d)]:
                    nc.sync.dma_start(
                        dst[:], srct.ap().rearrange("(t p) m -> p t m", p=128)
                    )
                xkvh_sb = xkv_pool.tile([128, KT, N], f16)
                kv_loaded = set()

                def kv_hi(t):
                    if t not in kv_loaded:
                        kv_loaded.add(t)
                        nc.sync.dma_start(
                            xkvh_sb[:, t, :],
                            xkvh_d.ap().rearrange("(t p) n -> p t n", p=128)[
                                :, t, :
                            ],
                        )
                    return xkvh_sb[:, t, :]

                def kv_lo(t):
                    xl = xp.tile([128, N], f16, tag="xlo")
                    nc.sync.dma_start(
                        xl[:], xkvl_d.ap().rearrange("(t p) n -> p t n", p=128)[:, t, :]
                    )
                    return xl

                proj_split_repack(
                    xp, xs_pool, kv_hi, kv_lo, wkh_sb, wkl_sb, kpk, hi_top=False
                )

                # V directly in [m, hv] layout: V = x_t.T @ wv (fp16)
                nc.sync.dma_start(
                    wv_sb[:], wv_d.ap().rearrange("(t p) m -> p t m", p=128)
                )
                with tc.tile_pool(name="pv", bufs=1, space="PSUM") as pv:
                    for half in range(2):
                        vps = [
                            pv.tile([128, 256], f32, tag=f"v{j}", name=f"v{j}")
                            for j in range(8)
                        ]
                        for t in range(KT):
                            for j in range(8):
                                mb = 8 * half + j
                                nc.tensor.matmul(
                                    vps[j][:],
                                    xkvh_sb[:, t, 128 * mb : 128 * (mb + 1)],
                                    wv_sb[:, t, :],
                                    start=(t == 0), stop=(t == KT - 1),
                                )
                        for j in range(8):
                            nc.vector.tensor_copy(V_sb[:, 8 * half + j, :], vps[j][:])
                nc.sync.dma_start(
                    wo_sb[:], wo_d.ap().rearrange("(g p) d -> p g d", p=128)
                )
                for j in range(NSCR):
                    pad_insts.append(
                        nc.gpsimd.dma_start(
                            AP(scrb[j], 2048, [[SPITCH, 128], [1, TPAD]]),
                            padc[:, :TPAD],
                        )
                    )
                for i in range(8):
                    W = 128 * (i + 1)
                    P = W + TPAD
                    pad_s_insts[i] = nc.gpsimd.dma_start(
                        AP(scrs[i], W, [[P, 128], [128 * P, 4], [1, TPAD]]),
                        padc[:],
                    )

            def strip_mms(i, h, tT_ap, col0, tps_pool):
                # diagonal-layout position strip matmuls for (h, block i) into
                # tT_ap columns [col0, col0+W)
                q_hh = qdup[64:128, h, :]
                q_pk = qpk[:, h, :]
                t_hh = tab_sb[64:128, h, :]
                t_pk = tab_sb[:, h, :]
                nb = 128 * i
                W = nb + 128
                nslc = slice(nb, nb + 128)
                for sc in range((W + 511) // 512):
                    wdt = min(512, W - 512 * sc)
                    tps = tps_pool.tile([128, 512], f32, tag="tps")
                    ts = slice(TW - W + 512 * sc, TW - W + 512 * sc + wdt)
                    nc.tensor.matmul(
                        tps[:, :wdt], q_hh[:, nslc], t_hh[:, ts],
                        start=True, stop=False,
                    )
                    nc.tensor.matmul(
                        tps[:, :wdt], q_pk[:, nslc], t_pk[:, ts],
                        start=False, stop=True,
                    )
                    cs = col0 + 512 * sc
                    nc.scalar.copy(tT_ap[:, cs : cs + wdt], tps[:, :wdt])

            def emit_strips(i, tps_pool, lpool, tpool):
                # returns {h: (Lp_tile, col0)}; batches 4/2/1 heads per DRAM
                # round trip depending on the strip width
                W = 128 * (i + 1)
                P = W + TPAD
                nbh = 4 if i < 4 else (2 if i < 8 else 1)
                out = {}
                for tb_ in range(HQ // nbh):
                    heads = [tb_ * nbh + j for j in range(nbh)]
                    tT = tpool.tile([128, 2048], f32, tag="tT")
                    for idx, h in enumerate(heads):
                        strip_mms(i, h, tT, idx * W, tps_pool)
                    if nbh == 1:
                        h = heads[0]
                        bj = next_scr[0] % NSCR
                        next_scr[0] += 1
                        s = scrb[bj]
                        Lp = lpool.tile([128, 2048], f32, tag="Lp")
                        # split the round trip in halves: the first fuse
                        # chunks start after only half the DMA latency
                        W2 = (W // 2 + 511) // 512 * 512
                        prev_reads = last_read[bj]
                        reads = []
                        w_insts = []
                        # writes overlap the read split by 128 cols: a read of
                        # [c0, c1) touches strip cols up to c1+127 (the shear)
                        wbounds = [0, min(W2 + 128, W)]
                        if wbounds[-1] < W:
                            wbounds.append(W)
                        for c0, c1 in zip(wbounds[:-1], wbounds[1:]):
                            if c1 <= c0:
                                continue
                            w_inst = nc.sync.dma_start(
                                AP(s, 2048 - W + c0, [[SPITCH, 128], [1, c1 - c0]]),
                                tT[:, c0:c1],
                            )
                            if prev_reads is not None:
                                for pr in prev_reads:
                                    add_dep_helper(
                                        w_inst.ins, pr.ins, reason="WAR"
                                    )
                            w_insts.append(w_inst)
                        for c0 in range(0, W, 512):
                            c1 = min(c0 + 512, W)
                            r_inst = nc.sync.dma_start(
                                Lp[:, c0:c1],
                                AP(
                                    s,
                                    2048 - W + 127 + c0,
                                    [[SPITCH - 1, 128], [1, c1 - c0]],
                                ),
                            )
                            for (wc0, wc1), wi in zip(
                                zip(wbounds[:-1], wbounds[1:]), w_insts
                            ):
                                if wc1 > c0 and wc0 < min(c1 + 128, W):
                                    add_dep_helper(
                                        r_inst.ins, wi.ins, reason="RAW"
                                    )
                            add_dep_helper(
                                r_inst.ins, pad_insts[bj].ins, reason="pad"
                            )
                            reads.append(r_inst)
                        last_read[bj] = reads
                        out[h] = (Lp, 0)
                    else:
                        s = scrs[i]
                        base = heads[0] * 128 * P
                        w_inst = nc.sync.dma_start(
                            AP(s, base, [[P, 128], [128 * P, nbh], [1, W]]),
                            tT[:, : nbh * W],
                        )
                        key = (i, tb_)
                        if last_read_s.get(key) is not None:
                            add_dep_helper(
                                w_inst.ins, last_read_s[key].ins, reason="WAR"
                            )
                        Lp = lpool.tile([128, 2048], f32, tag="Lp")
                        r_inst = nc.sync.dma_start(
                            Lp[:, : nbh * W],
                            AP(
                                s, base + 127,
                                [[P - 1, 128], [128 * P, nbh], [1, W]],
                            ),
                        )
                        add_dep_helper(r_inst.ins, w_inst.ins, reason="RAW")
                        add_dep_helper(
                            r_inst.ins, pad_s_insts[i].ins, reason="pad"
                        )
                        last_read_s[key] = r_inst
                        for idx, h in enumerate(heads):
                            out[h] = (Lp, idx * W)
                return out

            def phase_attn(pa, pb, pc, lpool, lpool3, tpool, spool):
                done = set()
                for i in range(16):
                    nb = 128 * i
                    W = nb + 128
                    nch = (W + 511) // 512
                    nslc = slice(nb, nb + 128)
                    lps = emit_strips(i, pa, lpool3, tpool)
                    for h in range(HQ):
                        q_hh = qdup[64:128, h, :]
                        q_pk = qpk[:, h, :]
                        k_hh = kpk[64:128, h, :]
                        k_pk = kpk[:, h, :]
                        Lp, col0 = lps[h]

                        # --- content logits + fused add/max ---
                        if i < 4:
                            L_sb = spool.tile([128, 512], f32, tag="Ls")
                        else:
                            L_sb = lpool.tile([128, 2048], f32, tag="L")
                        cmax = work.tile([128, 4], f32, tag="cmax")
                        for c in range(nch):
                            rw = min(512, W - 512 * c)
                            ms = slice(512 * c, 512 * c + rw)
                            cps = pb.tile([128, 512], f32, tag="cps")
                            nc.tensor.matmul(
                                cps[:, :rw], q_hh[:, nslc], k_hh[:, ms],
                                start=True, stop=False,
                            )
                            nc.tensor.matmul(
                                cps[:, :rw], q_pk[:, nslc], k_pk[:, ms],
                                start=False, stop=True,
                            )
                            nc.vector.tensor_add(
                                L_sb[:, ms],
                                cps[:, :rw],
                                Lp[:, col0 + 512 * c : col0 + 512 * c + rw],
                            )
                            if nch > 1:
                                nc.vector.reduce_max(
                                    out=cmax[:, c : c + 1], in_=L_sb[:, ms], axis=AX
                                )
                        negmax = work.tile([128, 1], f32, tag="negmax")
                        if nch == 1:
                            nc.vector.tensor_reduce(
                                out=negmax[:], in_=L_sb[:, :W], axis=AX, op=MAX,
                                negate=True,
                            )
                        else:
                            nc.vector.tensor_reduce(
                                out=negmax[:], in_=cmax[:, :nch], axis=AX, op=MAX,
                                negate=True,
                            )

                        # --- softmax ---
                        if i < 4:
                            P_all = spool.tile([128, 512], f16, tag="Ps")
                        else:
                            P_all = tpool.tile([128, 2048], f16, tag="Pall")
                        ssum = work.tile([128, 1], f32, tag="ssum")
                        nc.scalar.activation(
                            P_all[:, :W], L_sb[:, :W], EXP,
                            bias=negmax[:], scale=1.0, accum_out=ssum[:],
                        )
                        rsum = work.tile([128, 1], f32, tag="rsum")
                        nc.vector.reciprocal(rsum[:], ssum[:])
                        nc.vector.tensor_scalar_mul(
                            P_all[:, :W], P_all[:, :W], rsum[:]
                        )

                        # --- transpose P + AV ---
                        atps = pc.tile([64, 128], f32, tag="avp")
                        for tb in range((i + 8) // 8):
                            nsub = min(8, i + 1 - 8 * tb)
                            ptps = pa.tile([128, 1024], f16, tag="ptp")
                            for sb_ in range(nsub):
                                mt = 8 * tb + sb_
                                nc.tensor.transpose(
                                    ptps[:, 128 * sb_ : 128 * (sb_ + 1)],
                                    P_all[:, 128 * mt : 128 * (mt + 1)],
                                    ident16[:],
                                )
                            pt_sb = work.tile([128, 1024], f16, tag="pts")
                            if i >= 10 or (i + tb) % 2 == 0:
                                nc.scalar.copy(
                                    pt_sb[:, : 128 * nsub], ptps[:, : 128 * nsub]
                                )
                            else:
                                nc.vector.tensor_copy(
                                    pt_sb[:, : 128 * nsub], ptps[:, : 128 * nsub]
                                )
                            for sb_ in range(nsub):
                                mt = 8 * tb + sb_
                                nc.tensor.matmul(
                                    atps[:],
                                    V_sb[:, mt, 64 * h : 64 * (h + 1)],
                                    pt_sb[:, 128 * sb_ : 128 * (sb_ + 1)],
                                    start=(mt == 0), stop=(mt == i),
                                )
                        ro = 64 * (h % 2)
                        if i >= 10:
                            nc.scalar.copy(
                                attn16[ro : ro + 64, h // 2, nslc], atps[:]
                            )
                        else:
                            nc.vector.tensor_copy(
                                attn16[ro : ro + 64, h // 2, nslc], atps[:]
                            )
                    # incremental out projection per 512-column chunk
                    done.add(i)
                    nn = i // 4
                    if all(4 * nn + j in done for j in range(4)):
                        phase_out_chunk(pb, nn)

            def phase_out_chunk(pa, nn):
                ns = slice(512 * nn, 512 * (nn + 1))
                for dc in range(8):
                    ops = pa.tile([128, 512], f32, tag="cps")
                    dslc = slice(128 * dc, 128 * (dc + 1))
                    nc.tensor.matmul(
                        ops[:], wo_sb[:, 0, dslc], attn16[:, 0, ns],
                        start=True, stop=False,
                    )
                    nc.tensor.matmul(
                        ops[:], wo_sb[:, 1, dslc], attn16[:, 1, ns],
                        start=False, stop=True,
                    )
                    o_sb = work.tile([128, 512], f16, tag="osb")
                    nc.scalar.copy(o_sb[:], ops[:])
                    nc.sync.dma_start(outT.ap()[dc, :, ns], o_sb[:])

            def loop_body():
                with ExitStack() as phx:
                    xp = phx.enter_context(tc.tile_pool(name="xp", bufs=3))
                    xs_pool = phx.enter_context(tc.tile_pool(name="xs", bufs=1))
                    xkv_pool = phx.enter_context(tc.tile_pool(name="xkv", bufs=1))
                    phase_kqv(xp, xs_pool, xkv_pool)
                with ExitStack() as phx:
                    lpool = phx.enter_context(tc.tile_pool(name="lpool", bufs=2))
                    lpool3 = phx.enter_context(tc.tile_pool(name="lpool3", bufs=2))
                    tpool = phx.enter_context(tc.tile_pool(name="tpool", bufs=3))
                    pa = phx.enter_context(tc.tile_pool(name="pa", bufs=2, space="PSUM"))
                    pb = phx.enter_context(tc.tile_pool(name="pb", bufs=3, space="PSUM"))
                    pc = phx.enter_context(tc.tile_pool(name="pc", bufs=1, space="PSUM"))
                    spool = phx.enter_context(tc.tile_pool(name="spool", bufs=4))
                    phase_attn(pa, pb, pc, lpool, lpool3, tpool, spool)

            if loop == 1:
                loop_body()
            else:
                with tc.For_i(0, loop, 1):
                    loop_body()

    nc.compile()
    return nc


def _host_table():
    """Position table T[delta, h, dqk] for delta in [0, 2048), fp32 sincos
    pipeline (matching the reference) with an fp64 contraction."""
    r = np.arange(0.0, float(TW), dtype=np.float32)
    inv_freq = (
        1.0 / (10000.0 ** (np.arange(0.0, D, 2.0, dtype=np.float32) / np.float32(D)))
    ).astype(np.float32)
    phases = (r[:, None] * inv_freq[None, :]).astype(np.float32)
    sc = np.concatenate(
        [np.sin(phases, dtype=np.float32), np.cos(phases, dtype=np.float32)], axis=-1
    )
    return sc  # [TW, D]


def _split16(a):
    h = a.astype(np.float16)
    l = (a - h.astype(np.float32)).astype(np.float16)
    return h, l


def _prep_inputs(x_q, x_kv, to_q, to_kv, for_pos_enc, to_o):
    x_q = np.asarray(x_q, dtype=np.float32)
    x_kv = np.asarray(x_kv, dtype=np.float32)
    to_q = np.asarray(to_q, dtype=np.float32)
    to_kv = np.asarray(to_kv, dtype=np.float32)
    fpe = np.asarray(for_pos_enc, dtype=np.float32)
    to_o = np.asarray(to_o, dtype=np.float32)

    if "sc" not in _cache:
        _cache["sc"] = _host_table()
    sc = _cache["sc"]
    # T[delta, h, dqk] in fp64 then reversed over delta
    T = np.einsum("rb,hdb->rhd", sc.astype(np.float64), fpe.astype(np.float64))
    T = np.ascontiguousarray(T[::-1]).astype(np.float32)  # col jr <-> delta 2047-jr

    xs = {}
    for z in range(B):
        xs[("q", z)] = _split16(np.ascontiguousarray(x_q[:, z, :].T))
        xs[("kv", z)] = _split16(np.ascontiguousarray(x_kv[:, z, :].T))

    in_maps = []
    for c in range(NCORES):
        z = c // 4
        quad = c % 4
        hs = slice(HQ * quad, HQ * (quad + 1))
        wq = np.ascontiguousarray(to_q[hs].reshape(HQ * DQK, D).T)
        wk = np.ascontiguousarray(to_kv[hs, :DQK].reshape(HQ * DQK, D).T)
        wqh, wql = _split16(wq)
        wkh, wkl = _split16(wk)
        wv = np.ascontiguousarray(to_kv[hs, DQK:].reshape(HQ * DV, D).T).astype(
            np.float16
        )
        wo = np.ascontiguousarray(
            (to_o[:, hs, :] / 16.0).reshape(D, HQ * DV).T
        ).astype(np.float16)
        # packed table: per head rows [tabl(64); tabh(64)], cols reversed delta
        tab = np.empty((128, HQ * TW), dtype=np.float16)
        for h in range(HQ):
            Th, Tl = _split16(np.ascontiguousarray(T[:, HQ * quad + h, :].T))  # [64,TW]
            tab[0:64, TW * h : TW * (h + 1)] = Tl
            tab[64:128, TW * h : TW * (h + 1)] = Th
        xqh, xql = xs[("q", z)]
        xkvh, xkvl = xs[("kv", z)]
        in_maps.append(
            {
                "xqh": xqh, "xql": xql, "xkvh": xkvh, "xkvl": xkvl,
                "tabpk": tab,
                "wqh": wqh, "wql": wql, "wkh": wkh, "wkl": wkl,
                "wv": wv, "wo": wo,
            }
        )
    return in_maps


def kernel(x_q, x_kv, to_q, to_kv, for_pos_enc, to_o):
    from concourse.bass_utils import run_bass_kernel_spmd

    if "nc" not in _cache:
        _cache["nc"] = _build()
    nc = _cache["nc"]
    in_maps = _prep_inputs(x_q, x_kv, to_q, to_kv, for_pos_enc, to_o)
    res = run_bass_kernel_spmd(nc, in_maps, core_ids=list(range(NCORES)))
    out = np.zeros((B, D, N), dtype=np.float64)
    for c in range(NCORES):
        z = c // 4
        out[z] += res.results[c]["outT"].reshape(D, N).astype(np.float64)
    out *= 16.0
    return np.ascontiguousarray(out.transpose(2, 0, 1)).astype(np.float32)
